# revision 19
# baseline (speedup 1.0000x reference)
"""Multi-head attention (B=8, N=1024, C=1024, H=16) on 8 TRN2 NeuronCores.

Strategy: pure data parallelism — each core computes one batch element with
replicated weights (no collectives). All matmuls in float32r (full-rate
fp32, ~1e-3 operand rounding). Host pre-transposes weights/activations so
every matmul contracts over the partition axis.

Per-core pipeline:
  A: V natural [token, vfeat] in a 65-wide-per-head layout whose extra ones
     column makes the PV matmul emit softmax row-sums for free.
  Pair loop (hp = head pair, 8 iterations), fully pipelined:
     - just-in-time projection of this pair's q and k feature blocks
       (k block then q block) from streamed 256-col weight slices
     - per head: S^T[key, query] = K_h^T.T @ Q_h^T -> exp -> PV accumulate
       into O^T[65, N] (row 64 = softmax sums)
     - normalization: sums -> [128, 8] reshape (multi-lane reciprocal) ->
       partition 0 -> gpsimd partition_broadcast -> multiply; bounce-DMA
       the normalized head into the A^T tile (aliases the dead q tile)
  D: outT[co, token] = woT.T @ A^T + bias
"""
import numpy as np

B, N, C = 8, 1024, 1024
H = 16
HD = C // H               # 64
SCALE = HD ** (-0.5)
NCORES = 8

_COMPILED = {}


def _build():
    import concourse.bass as bass
    import concourse.tile as tile
    from concourse import bacc, mybir

    F32 = mybir.dt.float32
    F32R = mybir.dt.float32r
    EXP = mybir.ActivationFunctionType.Exp

    nc = bacc.Bacc("TRN2", target_bir_lowering=False, debug=False)

    xT = nc.dram_tensor("xT", [C, N], F32R, kind="ExternalInput").ap()
    wqkT = nc.dram_tensor("wqkT", [C, 2 * C], F32R, kind="ExternalInput").ap()
    wvT = nc.dram_tensor("wvT", [C, C], F32R, kind="ExternalInput").ap()
    woT = nc.dram_tensor("woT", [C, C], F32R, kind="ExternalInput").ap()
    bqk = nc.dram_tensor("bqk", [128, 16], F32, kind="ExternalInput").ap()
    bv = nc.dram_tensor("bv", [1, C], F32R, kind="ExternalInput").ap()
    bo = nc.dram_tensor("bo", [128, 8], F32, kind="ExternalInput").ap()
    ones_col = nc.dram_tensor("ones_col", [128, 16], F32R, kind="ExternalInput").ap()
    ones_row = nc.dram_tensor("ones_row", [1, 512], F32R, kind="ExternalInput").ap()
    outT = nc.dram_tensor("outT", [C, N], F32, kind="ExternalOutput").ap()

    CB = C // 128      # 8 contraction blocks
    TB = N // 128      # 8 token blocks
    VW = 65            # per-head V width (64 feats + ones col)

    # wqkT viewed as [c, 16 feature-blocks, 128] for pair-sliced weight loads
    wqk3 = wqkT.rearrange("c (g j) -> c g j", g=16)

    with tile.TileContext(nc) as tc:
        with tc.tile_pool(name="misc", bufs=1) as pool_misc, \
             tc.tile_pool(name="V", bufs=1) as pool_V, \
             tc.tile_pool(name="qA", bufs=1) as pool_qA, \
             tc.tile_pool(name="x", bufs=1) as pool_x, \
             tc.tile_pool(name="wo", bufs=1) as pool_wo:

            bqk_sb = pool_misc.tile([128, 16], F32, tag="bqk")
            bv_sb = pool_misc.tile([1, C], F32R, tag="bv")
            bo_sb = pool_misc.tile([128, 8], F32, tag="bo")
            ones_sb = pool_misc.tile([1, 512], F32R, tag="ones")

            V_sb = [pool_V.tile([128, H * VW], F32R, tag=f"V{tb}", name=f"V{tb}")
                    for tb in range(TB)]
            # q-feature blocks; later overwritten in place with normalized A^T
            qA_sb = [pool_qA.tile([128, N], F32R, tag=f"qA{hp}", name=f"qA{hp}")
                     for hp in range(8)]
            x_sb = [pool_x.tile([128, N], F32R, tag=f"x{cb}", name=f"x{cb}")
                    for cb in range(CB)]
            wo_sb = [pool_wo.tile([128, C], F32R, tag=f"wo{cb}", name=f"wo{cb}")
                     for cb in range(CB)]

            # x first (A and the pair loop both need it), then small consts
            for cb in range(CB):
                for ch in range(2):
                    nc.sync.dma_start(
                        x_sb[cb][:, ch * 512:(ch + 1) * 512],
                        xT[cb * 128:(cb + 1) * 128, ch * 512:(ch + 1) * 512])
            nc.sync.dma_start(bqk_sb[:, :], bqk)
            nc.sync.dma_start(bv_sb[:, :], bv)
            nc.sync.dma_start(bo_sb[:, :], bo)
            nc.sync.dma_start(ones_sb[:, :], ones_row)

            # ---- A: V natural [token, vfeat] ----
            with tc.tile_pool(name="wv", bufs=1) as pool_wv, \
                 tc.tile_pool(name="ps_A", bufs=4, space="PSUM") as ps_A:
                wv_sb = [pool_wv.tile([128, C], F32R, tag=f"wv{cb}", name=f"wv{cb}")
                         for cb in range(CB)]
                for cb in range(CB):
                    for ch in range(2):
                        nc.sync.dma_start(
                            wv_sb[cb][:, ch * 512:(ch + 1) * 512],
                            wvT[cb * 128:(cb + 1) * 128, ch * 512:(ch + 1) * 512])
                for tb in range(TB):
                    nc.sync.dma_start(V_sb[tb][:, 64::VW], ones_col)
                for tb in range(TB):
                    for vc in range(2):
                        ps = ps_A.tile([128, 512], F32, tag="psA")
                        for cb in range(CB):
                            nc.tensor.matmul(
                                ps[:, :],
                                x_sb[cb][:, tb * 128:(tb + 1) * 128],
                                wv_sb[cb][:, vc * 512:(vc + 1) * 512],
                                start=(cb == 0), stop=False,
                            )
                        nc.tensor.matmul(
                            ps[:, :],
                            ones_sb[0:1, 0:128],
                            bv_sb[0:1, vc * 512:(vc + 1) * 512],
                            start=False, stop=True,
                        )
                        # scatter 8 heads x 64 cols into the 65-strided layout
                        dst = V_sb[tb][:, vc * 8 * VW:(vc + 1) * 8 * VW]
                        dst3 = dst.rearrange("p (h d) -> p h d", h=8)[:, :, 0:64]
                        src3 = ps[:, :].rearrange("p (h d) -> p h d", h=8)
                        nc.vector.tensor_copy(dst3, src3)

            # ---- fused pair loop: project q/k then attention ----
            with tc.tile_pool(name="wqkp", bufs=24) as pool_wqkp, \
                 tc.tile_pool(name="kblk", bufs=3) as pool_kblk, \
                 tc.tile_pool(name="PT", bufs=4) as pool_PT, \
                 tc.tile_pool(name="norm", bufs=1) as pool_norm, \
                 tc.tile_pool(name="ps_p", bufs=1, space="PSUM") as ps_p, \
                 tc.tile_pool(name="ps_S", bufs=2, space="PSUM") as ps_S, \
                 tc.tile_pool(name="ps_O", bufs=3, space="PSUM") as ps_O:

                for hp in range(8):
                    # -- stream this pair's weight slices: [c, {q|k} block] --
                    wq_t = []
                    for cb in range(CB):
                        t = pool_wqkp.tile([128, 256], F32R, tag="wqkp", name="wqkp")
                        src = wqk3[cb * 128:(cb + 1) * 128, hp::8, :]
                        nc.sync.dma_start(t[:, :].rearrange("p (g j) -> p g j", g=2),
                                          src)
                        wq_t.append(t)

                    # -- project k block (cols 128:256) then q block (0:128) --
                    kt = pool_kblk.tile([128, N], F32R, tag="kblk", name="kblk")
                    for dst_t, col0, bcol in ((kt, 128, 8 + hp), (qA_sb[hp], 0, hp)):
                        for nch in range(2):
                            ps = ps_p.tile([128, 512], F32, tag="psP")
                            for cb in range(CB):
                                nc.tensor.matmul(
                                    ps[:, :],
                                    wq_t[cb][:, col0:col0 + 128],
                                    x_sb[cb][:, nch * 512:(nch + 1) * 512],
                                    start=(cb == 0), stop=(cb == CB - 1),
                                )
                            nc.vector.tensor_scalar(
                                dst_t[:, nch * 512:(nch + 1) * 512], ps[:, :],
                                bqk_sb[:, bcol:bcol + 1], None, mybir.AluOpType.add,
                            )

                    # -- attention, one head at a time --
                    a_tmp = {}
                    for hh in range(2):
                        h = 2 * hp + hh
                        r0, r1 = hh * 64, hh * 64 + 64
                        o_ps = {ic: ps_O.tile([VW, 512], F32, tag="O", name="O")
                                for ic in range(2)}
                        for kb in range(TB):
                            s_ps = ps_S.tile([128, N], F32, tag="S")
                            for ic in range(2):
                                nc.tensor.matmul(
                                    s_ps[:, ic * 512:(ic + 1) * 512],
                                    kt[r0:r1, kb * 128:(kb + 1) * 128],
                                    qA_sb[hp][r0:r1, ic * 512:(ic + 1) * 512],
                                    start=True, stop=True,
                                )
                            p_t = pool_PT.tile([128, N], F32R, tag="pt")
                            nc.scalar.activation(p_t[:, :], s_ps[:, :], EXP,
                                                 scale=float(SCALE))
                            for ic in range(2):
                                nc.tensor.matmul(
                                    o_ps[ic][:, :],
                                    V_sb[kb][:, h * VW:(h + 1) * VW],
                                    p_t[:, ic * 512:(ic + 1) * 512],
                                    start=(kb == 0), stop=(kb == TB - 1),
                                )
                        # normalization: sums -> fast reciprocal -> broadcast
                        s_hi = pool_norm.tile([VW, N], F32, tag="shi")
                        for ic in range(2):
                            nc.vector.tensor_copy(
                                s_hi[64:65, ic * 512:(ic + 1) * 512],
                                o_ps[ic][64:65, :])
                        s128 = pool_norm.tile([128, 8], F32, tag="s128")
                        nc.sync.dma_start(s128[:, :], s_hi[64:65, :])
                        r128 = pool_norm.tile([128, 8], F32, tag="r128")
                        nc.vector.reciprocal(r128[:, :], s128[:, :])
                        r0t = pool_norm.tile([1, N], F32, tag="r0")
                        nc.sync.dma_start(r0t[0:1, :], r128[:, :])
                        r_rep = pool_norm.tile([64, N], F32, tag="rrep")
                        nc.gpsimd.partition_broadcast(r_rep[:, :], r0t[0:1, :])
                        a_tmp[hh] = pool_norm.tile([64, N], F32R, tag=f"atmp{hh}",
                                                   name=f"atmp{hh}")
                        for ic in range(2):
                            nc.vector.tensor_mul(
                                a_tmp[hh][:, ic * 512:(ic + 1) * 512],
                                o_ps[ic][0:64, :],
                                r_rep[:, ic * 512:(ic + 1) * 512])
                    # A^T lands in the dead q tile only after both heads'
                    # S reads are done (avoids any false write-after-read)
                    for hh in range(2):
                        for ch in range(2):
                            nc.sync.dma_start(
                                qA_sb[hp][hh * 64:hh * 64 + 64,
                                          ch * 512:(ch + 1) * 512],
                                a_tmp[hh][:, ch * 512:(ch + 1) * 512])

            # wo loads overlap the pair loop; issued here but dependency-free
            for cb in range(CB):
                for ch in range(2):
                    nc.sync.dma_start(
                        wo_sb[cb][:, ch * 512:(ch + 1) * 512],
                        woT[cb * 128:(cb + 1) * 128, ch * 512:(ch + 1) * 512])

            # ---- D: out projection ----
            with tc.tile_pool(name="outp", bufs=3) as pool_out, \
                 tc.tile_pool(name="ps_out", bufs=4, space="PSUM") as ps_out:
                for cb in range(CB):
                    for nch in range(2):
                        ps = ps_out.tile([128, 512], F32, tag="po")
                        for hb in range(8):
                            nc.tensor.matmul(
                                ps[:, :],
                                wo_sb[hb][:, cb * 128:(cb + 1) * 128],
                                qA_sb[hb][:, nch * 512:(nch + 1) * 512],
                                start=(hb == 0), stop=(hb == 7),
                            )
                        o_t = pool_out.tile([128, 512], F32, tag="ot")
                        nc.vector.tensor_scalar(
                            o_t[:, :], ps[:, :], bo_sb[:, cb:cb + 1], None,
                            mybir.AluOpType.add,
                        )
                        nc.sync.dma_start(
                            outT[cb * 128:(cb + 1) * 128,
                                 nch * 512:(nch + 1) * 512],
                            o_t[:, :],
                        )
    nc.compile()
    return nc


def _get_nc():
    if "nc" not in _COMPILED:
        _COMPILED["nc"] = _build()
    return _COMPILED["nc"]


def _run(x, in_proj_weight, in_proj_bias, out_proj_weight, out_proj_bias,
         trace=False):
    from concourse.bass_utils import run_bass_kernel_spmd

    nc = _get_nc()
    x = np.ascontiguousarray(np.asarray(x, dtype=np.float32))
    w_in = np.asarray(in_proj_weight, dtype=np.float32)
    b_in = np.asarray(in_proj_bias, dtype=np.float32)
    w_out = np.asarray(out_proj_weight, dtype=np.float32)
    b_out = np.asarray(out_proj_bias, dtype=np.float32)

    wqkT = np.ascontiguousarray(w_in[0:2 * C].T)          # [C, 2C]
    wvT = np.ascontiguousarray(w_in[2 * C:3 * C].T)       # [C, C]
    woT = np.ascontiguousarray(w_out.T)                   # [C, C]
    shared = {
        "wqkT": wqkT,
        "wvT": wvT,
        "woT": woT,
        "bqk": np.ascontiguousarray(b_in[0:2 * C].reshape(16, 128).T),
        "bv": np.ascontiguousarray(b_in[2 * C:3 * C])[None, :],
        "bo": np.ascontiguousarray(b_out.reshape(8, 128).T),
        "ones_col": np.ones((128, 16), dtype=np.float32),
        "ones_row": np.ones((1, 512), dtype=np.float32),
    }
    in_maps = []
    for c in range(NCORES):
        m = dict(shared)
        m["xT"] = np.ascontiguousarray(x[c].T)
        in_maps.append(m)

    res = run_bass_kernel_spmd(nc, in_maps, core_ids=list(range(NCORES)),
                               trace=trace)
    out = np.stack([
        np.ascontiguousarray(res.results[c]["outT"].T) for c in range(NCORES)
    ]).astype(np.float32)
    return out, res


def kernel(x, in_proj_weight, in_proj_bias, out_proj_weight, out_proj_bias):
    out, _ = _run(x, in_proj_weight, in_proj_bias, out_proj_weight,
                  out_proj_bias)
    return out
